# revision 1
# baseline (speedup 1.0000x reference)
"""Trainium2 Bass kernel for BaseRelationNetwork forward pass.

Reference computation (per batch row b):
    pairs (i<j) of C=16 channels, P=120 pairs
    h1 = relu(concat(x_i, x_j) @ W1 + b1)      # W1 [2F, H]
    h2 = relu(h1 @ W2 + b2)
    out = mean_p(h2 @ W3 + b3)                 # [B, H]

Algebraic restructuring used here:
  1. W1 splits into W1a (top F rows, applied to x_i) and W1b (bottom F rows,
     applied to x_j). ya = x @ W1a and yb = x @ W1b are computed once per
     channel (C matmuls) instead of per pair (P matmuls): 7.5x less PE work.
     h1[p=(i,j)] = relu(ya[i] + yb[j] + b1) is a cheap DVE gather-add.
  2. mean over pairs commutes with the affine layer 3:
     out = (mean_p h2) @ W3 + b3. Layer 3 runs on the pair-mean only.

Sharding: data-parallel over batch. 512 rows / 8 cores = 64 rows per core.
Weights replicated. Host pre-transposes x to feature-major layout with
token = half*512 + c*32 + b (batch split in two halves of 32) so the
pipeline (layer-1 matmul -> pair-add -> layer-2 -> accumulate) runs as two
overlapping chunks; the 1/P mean scale is folded into W3 and the biases
are packed into one [128, 6] tile on the host.

Matmuls run in float32r (fast fp32 mode, reduced mantissa): full PE rate
when the moving free dim >= 256, ~1e-4 output error vs exact fp32.

DMA strategy: big loads (x, W1) go through gpsimd (SWDGE) as a few large
multi-tile transfers - the HWDGE queue serializes ~0.6us per dma_start, so
many small sync-engine DMAs throttle the front of the kernel.
"""

import contextlib
import sys

if "/opt/trn_rl_repo" not in sys.path:
    sys.path.insert(0, "/opt/trn_rl_repo")

import numpy as np

import concourse.bass as bass
import concourse.mybir as mybir
import concourse.tile as tile
from concourse import bacc
from concourse.bass_utils import run_bass_kernel_spmd

# Problem shape (hardcoded per contract).
B, C, F, H = 512, 16, 1024, 256
N_CORES = 8
BL = B // N_CORES          # 64 local batch rows per core
P = C * (C - 1) // 2       # 120 pairs
NH = 4                     # batch chunks per core (chunked pipeline)
BH = BL // NH              # 32 rows per half
TOK = BL * C               # 1024 tokens per core
HTOK = BH * C              # 512 tokens per half, token = half*512 + c*32 + b
F32 = mybir.dt.float32
F32R = mybir.dt.float32r

KT1 = F // 128             # 8 k-tiles for layer-1 contraction
KQ = 4                     # k-tiles per merged x DMA
PPG = 30                   # pairs per stage-C sub-group
GW = PPG * BH              # stage-C sub-group width: 480 columns
NG = P // PPG              # 8 stage-C sub-groups per half
NSP = NG // 2              # 4 double-width (960-col) stage-C groups per half

# pair enumeration: for i in 0..C-2, j in i+1..C-1, p consecutive
PAIR_BASE = [0] * C
for _i in range(1, C):
    PAIR_BASE[_i] = PAIR_BASE[_i - 1] + (C - 1 - (_i - 1))

AF = mybir.ActivationFunctionType
ALU = mybir.AluOpType


def build_module(loop_iters: int = 1):
    nc = bacc.Bacc("TRN2", target_bir_lowering=False, debug=True)

    xt_d = nc.dram_tensor("xt", [F, TOK], F32R, kind="ExternalInput")
    w1_d = nc.dram_tensor("w1", [2 * F, H], F32R, kind="ExternalInput")
    w2_d = nc.dram_tensor("w2", [H, H], F32R, kind="ExternalInput")
    w3_d = nc.dram_tensor("w3", [H, H], F32, kind="ExternalInput")
    bp_d = nc.dram_tensor("bias_pack", [128, 6], F32, kind="ExternalInput")
    id_d = nc.dram_tensor("ident", [128, 128], F32R, kind="ExternalInput")
    out_d = nc.dram_tensor("outT", [H, BL], F32, kind="ExternalOutput")

    with tile.TileContext(nc) as tc:
        with (
            tc.tile_pool(name="xpool", bufs=1) as xpool,
            tc.tile_pool(name="wpool", bufs=1) as wpool,
            tc.tile_pool(name="ypool", bufs=1) as ypool,
            tc.tile_pool(name="hpool", bufs=1) as hpool,
            tc.tile_pool(name="spool", bufs=1) as spool,
            tc.tile_pool(name="psA", bufs=4, space="PSUM") as psA_pool,
            tc.tile_pool(name="psC", bufs=2, space="PSUM") as psC_pool,
        ):
            loop_cm = (
                tc.For_i(0, loop_iters, 1)
                if loop_iters > 1
                else contextlib.nullcontext()
            )
            with loop_cm:
                # big tiles
                xts = xpool.tile([128, KT1, TOK], F32R, tag="xts", name="xts")
                w1big = wpool.tile([128, 2 * KT1, H], F32R, tag="w1big", name="w1big")
                w2t = wpool.tile([128, 2, H], F32R, tag="w2t", name="w2t")
                w3t = wpool.tile([128, 2, H], F32, tag="w3t", name="w3t")
                bp = wpool.tile([128, 6], F32, tag="bp", name="bp")
                idt = wpool.tile([128, 128], F32R, tag="idt", name="idt")
                # y_all free layout: [m(4), chunk(NH), c(C), b(BH)]
                y_all = ypool.tile([128, 4, TOK], F32, tag="y_all", name="y_all")
                # h1 free layout: [t(2), half(NH), p(P), b(BH)]
                h1all = hpool.tile(
                    [128, 2, NH * P * BH], F32R, tag="h1all", name="h1all"
                )
                h2sb = [
                    [
                        spool.tile(
                            [128, GW * NG], F32R,
                            tag=f"h2_{m}_{par}", name=f"h2_{m}_{par}",
                        )
                        for par in range(2)
                    ]
                    for m in range(2)
                ]
                m2 = [
                    spool.tile([128, BL], F32, tag=f"m2_{m}", name=f"m2_{m}")
                    for m in range(2)
                ]
                osb = spool.tile([128, 2, BL], F32, tag="osb", name="osb")

                def bias(nm, t):
                    idx = {"b1": 0, "b2": 2, "b3": 4}[nm] + t
                    return bp[:, idx : idx + 1]

                # W1 rows viewed [16 ktiles, 128, H] -> SBUF [128, k, H]
                w1v = w1_d.rearrange("(k p) h -> p k h", p=128)
                xtv = xt_d.rearrange("(k p) t -> p k t", p=128)

                def hs(half):
                    return slice(half * HTOK, (half + 1) * HTOK)

                # ---- DMA order: bias first (ya copies need b1), W1 quads +
                # x chunk 0, then w2 (stage C), remaining x chunks, w3 last ----
                nc.sync.dma_start(out=bp[:], in_=bp_d[:])
                nc.sync.dma_start(out=idt[:], in_=id_d[:])
                for q in range(2):
                    ks = slice(q * KQ, (q + 1) * KQ)
                    kbs = slice(KT1 + q * KQ, KT1 + (q + 1) * KQ)
                    nc.gpsimd.dma_start(out=w1big[:, ks, :], in_=w1v[:, ks, :])
                    nc.gpsimd.dma_start(out=w1big[:, kbs, :], in_=w1v[:, kbs, :])
                    nc.gpsimd.dma_start(
                        out=xts[:, ks, hs(0)], in_=xtv[:, ks, hs(0)]
                    )
                nc.sync.dma_start(
                    out=w2t[:], in_=w2_d.rearrange("(k p) h -> p k h", p=128)
                )
                for ch in range(1, NH):
                    nc.gpsimd.dma_start(
                        out=xts[:, :, hs(ch)], in_=xtv[:, :, hs(ch)]
                    )
                nc.sync.dma_start(
                    out=w3t[:], in_=w3_d.rearrange("(k p) h -> p k h", p=128)
                )

                def flush_acc(p):
                    ph, ppar = p
                    for m in range(2):
                        # sum the 4 su-blocks on PE: identity pass-through
                        # matmuls accumulating in PSUM (PE has slack)
                        psr = psC_pool.tile(
                            [128, GW], F32, tag="psC", name=f"psR_{ph}_{m}"
                        )
                        for su in range(2 * NSP):
                            nc.tensor.matmul(
                                psr[:],
                                idt[:],
                                h2sb[m][ppar][:, su * GW : (su + 1) * GW],
                                start=(su == 0),
                                stop=(su == 2 * NSP - 1),
                            )
                        # then reduce over p only: [128, b, p] view, 480 reads
                        v = psr.rearrange("q (pp b) -> q pp b", b=BH).transpose(
                            [0, 2, 1]
                        )
                        nc.vector.tensor_reduce(
                            m2[m][:, ph * BH : (ph + 1) * BH],
                            v,
                            mybir.AxisListType.X,
                            ALU.add,
                        )

                # PE warm-up while DMAs stream: ~10 dummy matmuls on the bias
                # tile into a psC-pool slot (free until stage C starts ~18us)
                warm = psC_pool.tile([128, 1024], F32, tag="psC", name="warm")
                for _ in range(10):
                    nc.tensor.matmul(
                        warm[:1, :256],
                        bp[:, 0:1],
                        bp[:, 0:1].broadcast_to([128, 256]),
                        start=True,
                        stop=True,
                    )

                pend = None
                for half in range(NH):
                    # ---- stage A (k-outer): matmuls for this half ----
                    psA = {
                        m: psA_pool.tile(
                            [128, HTOK], F32, tag="psA", name=f"psA_{half}_{m}"
                        )
                        for m in range(4)
                    }
                    for k in range(KT1):
                        for m in (0, 2, 1, 3):
                            w_half, ht = divmod(m, 2)
                            nc.tensor.matmul(
                                psA[m][:],
                                w1big[:, w_half * KT1 + k, ht * 128 : (ht + 1) * 128],
                                xts[:, k, hs(half)],
                                start=(k == 0),
                                stop=(k == KT1 - 1),
                            )
                    # PSUM -> SBUF copies, split DVE/ACT; b1 folded into ya
                    for m in (0, 2, 1, 3):
                        if m < 2:
                            nc.vector.tensor_scalar_add(
                                y_all[:, m, hs(half)], psA[m][:], bias("b1", m)
                            )
                        else:
                            nc.scalar.copy(y_all[:, m, hs(half)], psA[m][:])

                    # ---- stage B: pair-add + bias on DVE, relu on ACT ----
                    # y_all viewed [128, m, half, c, b]; h1all [128, t, half, p, b]
                    hbase = half * P * BH
                    y5 = y_all.rearrange("p m (hh c b) -> p m hh c b", hh=NH, b=BH)
                    h5 = h1all.rearrange("p t (hh pp b) -> p t hh pp b", hh=NH, b=BH)
                    for i in range(C - 1):
                        nj = C - 1 - i
                        p0 = PAIR_BASE[i]
                        in0 = y5[:, 0:2, half, i : i + 1, :].broadcast_to(
                            [128, 2, nj, BH]
                        )
                        in1 = y5[:, 2:4, half, i + 1 :, :]
                        outap = h5[:, :, half, p0 : p0 + nj, :]
                        nc.vector.tensor_add(outap, in0, in1)
                    # relu in place, both t at once, 960-wide slices (ACT)
                    for sp in range(NSP):
                        sl = h1all[
                            :, :, hbase + sp * 2 * GW : hbase + (sp + 1) * 2 * GW
                        ]
                        nc.scalar.activation(sl, sl, AF.Relu)

                    # flush the PREVIOUS chunk's DVE accumulate chain now, so
                    # this chunk's pair-adds (above) fed PE/ACT first
                    if pend is not None:
                        flush_acc(pend)
                        pend = None

                    # ---- stage C+D: layer-2 matmul, relu(+b2) on ACT ----
                    par = half % 2
                    for sp in range(NSP):
                        for m in range(2):
                            ps = psC_pool.tile(
                                [128, 1024], F32, tag="psC",
                                name=f"psC_{half}_{m}_{sp}",
                            )
                            for sub in range(2):
                                s = sp * 2 + sub
                                for k in range(2):
                                    nc.tensor.matmul(
                                        ps[:, sub * 512 : sub * 512 + GW],
                                        w2t[:, k, m * 128 : (m + 1) * 128],
                                        h1all[
                                            :,
                                            k,
                                            hbase + s * GW : hbase + (s + 1) * GW,
                                        ],
                                        start=(k == 0),
                                        stop=(k == 1),
                                    )
                            h2t = h2sb[m][par][
                                :, sp * 2 * GW : (sp + 1) * 2 * GW
                            ].rearrange("p (u g) -> p u g", g=GW)
                            psv = ps.rearrange("p (u g) -> p u g", g=512)[:, :, :GW]
                            nc.scalar.activation(h2t, psv, AF.Relu, bias=bias("b2", m))
                    pend = (half, par)

                if pend is not None:
                    flush_acc(pend)
                    pend = None

                # ---- stage E: outT = (m2 @ W3scaled) + b3 (bias on DVE) ----
                for mo in range(2):
                    ps = psA_pool.tile([128, HTOK], F32, tag="psA", name=f"psE_{mo}")
                    po = ps[:, :BL]
                    for k in range(2):
                        nc.tensor.matmul(
                            po,
                            w3t[:, k, mo * 128 : (mo + 1) * 128],
                            m2[k][:],
                            start=(k == 0),
                            stop=(k == 1),
                        )
                    nc.vector.tensor_scalar_add(osb[:, mo, :], po, bias("b3", mo))
                nc.sync.dma_start(
                    out=out_d.rearrange("(m p) b -> p m b", p=128), in_=osb[:]
                )

    nc.compile()
    return nc


_NC_CACHE = None


def _get_module():
    global _NC_CACHE
    if _NC_CACHE is None:
        _NC_CACHE = build_module()
    return _NC_CACHE


def make_in_maps(x, W1, b1, W2, b2, W3, b3):
    W1 = np.ascontiguousarray(W1, dtype=np.float32)
    w3p = np.ascontiguousarray(W3, dtype=np.float32) / np.float32(P)
    b1 = np.asarray(b1, dtype=np.float32)
    b2 = np.asarray(b2, dtype=np.float32)
    b3 = np.asarray(b3, dtype=np.float32)
    bias_pack = np.stack(
        [b1[:128], b1[128:], b2[:128], b2[128:], b3[:128], b3[128:]], axis=1
    )
    bias_pack = np.ascontiguousarray(bias_pack, dtype=np.float32)
    in_maps = []
    for i in range(N_CORES):
        xs = x[i * BL : (i + 1) * BL]  # [BL, C, F]
        halves = [
            xs[h * BH : (h + 1) * BH].transpose(1, 0, 2).reshape(HTOK, F)
            for h in range(NH)
        ]
        xT = np.ascontiguousarray(np.concatenate(halves, axis=0).T, dtype=np.float32)
        in_maps.append(
            {
                "xt": xT,
                "w1": W1,
                "w2": np.ascontiguousarray(W2, dtype=np.float32),
                "w3": np.ascontiguousarray(w3p, dtype=np.float32),
                "bias_pack": bias_pack,
                "ident": np.eye(128, dtype=np.float32),
            }
        )
    return in_maps


def kernel(x, W1, b1, W2, b2, W3, b3):
    nc = _get_module()
    in_maps = make_in_maps(
        np.asarray(x, dtype=np.float32),
        np.asarray(W1),
        np.asarray(b1),
        np.asarray(W2),
        np.asarray(b2),
        np.asarray(W3),
        np.asarray(b3),
    )
    res = run_bass_kernel_spmd(nc, in_maps, list(range(N_CORES)))
    out = np.empty((B, H), dtype=np.float32)
    for i in range(N_CORES):
        out[i * BL : (i + 1) * BL] = res.results[i]["outT"].T
    return out



# revision 2
# speedup vs baseline: 1.1744x; 1.1744x over previous
"""Trainium2 Bass kernel for BaseRelationNetwork forward pass (v2).

Reference computation (per batch row b):
    pairs (i<j) of C=16 channels, P=120 pairs
    h1 = relu(concat(x_i, x_j) @ W1 + b1)      # W1 [2F, H]
    h2 = relu(h1 @ W2 + b2)
    out = mean_p(h2 @ W3 + b3)                 # [B, H]

Structure (same algebra as v1):
  ya = x @ W1a, yb = x @ W1b once per channel; h1[p=(i,j)] = relu(ya_i+yb_j+b1)
  mean commutes with layer 3: out = (mean_p h2) @ (W3/P) + b3.

v2 changes, driven by the TimelineSim cost model:
  * Stage-A matmuls run in fp8-e4m3 DoubleRow mode (0.5 cy/row): W1 is
    host-prescaled by 32 (fp8 mantissa headroom), descaled in the PSUM
    eviction via the ACT scale port. Measured end-to-end rel-err ~6e-3.
  * Everything else is fp16: halves DMA bytes and unlocks DVE 2x (TensorTensor
    pair-add) and 4x (TensorScalarPtr relu+bias) perf modes.
  * b1 is folded into the h1 relu pass (DVE tensor_scalar add+max), so the
    PSUM->SBUF y eviction is a plain ACT copy.
  * NH=2 batch halves (32 rows each); within a half the pair-add -> relu ->
    layer-2 -> evict -> pair-sum chain is pipelined at 30-pair groups.
  * The pair-sum runs as identity matmuls accumulating into a PSUM tile
    (interleaved with stage-C matmuls), then one DVE XY-reduce per (half, m).

Sharding: data-parallel over batch, 64 rows/core, weights replicated.
"""

import contextlib
import sys

if "/opt/trn_rl_repo" not in sys.path:
    sys.path.insert(0, "/opt/trn_rl_repo")

import numpy as np
import ml_dtypes

import concourse.bass as bass
import concourse.mybir as mybir
import concourse.tile as tile
from concourse import bacc
from concourse.bass_utils import run_bass_kernel_spmd

# Problem shape (hardcoded per contract).
B, C, F, H = 512, 16, 1024, 256
N_CORES = 8
BL = B // N_CORES          # 64 local batch rows per core
P = C * (C - 1) // 2       # 120 pairs
NH = 2                     # batch halves per core
BH = BL // NH              # 32 rows per half
TOK = BL * C               # 1024 tokens per core
HTOK = BH * C              # 512 tokens per half; token = half*512 + c*32 + b
PPG = 30                   # pairs per stage-C group
NG = P // PPG              # 4 stage-C groups per half
GW = PPG * BH              # 960 columns per group
SUBW = GW // 2             # 480 columns per psum sub-group (fits a bank)
KT8 = F // 256             # 4 DoubleRow k-tiles for layer-1
W1SCALE = 32.0             # host-side fp8 mantissa prescale on W1

F32 = mybir.dt.float32
F16 = mybir.dt.float16
F8 = mybir.dt.float8e4
DR = mybir.MatmulPerfMode.DoubleRow

# pair enumeration: for i in 0..C-2, j in i+1..C-1, p consecutive
PAIR_BASE = [0] * C
for _i in range(1, C):
    PAIR_BASE[_i] = PAIR_BASE[_i - 1] + (C - 1 - (_i - 1))

AF = mybir.ActivationFunctionType
ALU = mybir.AluOpType
AX = mybir.AxisListType

N_WARM = 13                # PE ramp-up matmuls while input DMAs stream


def build_module(loop_iters: int = 1):
    nc = bacc.Bacc("TRN2", target_bir_lowering=False, debug=True)

    # host pre-arranges x and W1 into SBUF layout (partition-major):
    # xt[p, (kt i t)] with feature f = kt*256 + i*128 + p
    # w1[p, (ab kt i h)]
    xt_d = nc.dram_tensor("xt", [128, KT8 * 2 * TOK], F8, kind="ExternalInput")
    w1_d = nc.dram_tensor("w1", [128, 2 * KT8 * 2 * H], F8, kind="ExternalInput")
    w2_d = nc.dram_tensor("w2", [H, H], F16, kind="ExternalInput")
    w3_d = nc.dram_tensor("w3", [H, H], F32, kind="ExternalInput")
    bp_d = nc.dram_tensor("bias_pack", [128, 6], F32, kind="ExternalInput")
    # double-stacked identity for the DoubleRow pair-sum flush
    id_d = nc.dram_tensor("ident2", [128, 2 * 128], F8, kind="ExternalInput")
    out_d = nc.dram_tensor("outT", [H, BL], F32, kind="ExternalOutput")

    with tile.TileContext(nc) as tc:
        with (
            tc.tile_pool(name="xpool", bufs=1) as xpool,
            tc.tile_pool(name="wpool", bufs=1) as wpool,
            tc.tile_pool(name="ypool", bufs=1) as ypool,
            tc.tile_pool(name="hpool", bufs=1) as hpool,
            tc.tile_pool(name="spool", bufs=1) as spool,
            tc.tile_pool(name="psA", bufs=2, space="PSUM") as psA_pool,
            tc.tile_pool(name="psR", bufs=2, space="PSUM") as psR_pool,
            tc.tile_pool(name="psC", bufs=2, space="PSUM") as psC_pool,
        ):
            loop_cm = (
                tc.For_i(0, loop_iters, 1)
                if loop_iters > 1
                else contextlib.nullcontext()
            )
            with loop_cm:
                # SBUF tiles
                # x: [p, kt, i, tok] with feature f = kt*256 + i*128 + p
                xts = xpool.tile([128, KT8, 2, TOK], F8, tag="xts", name="xts")
                # W1: [p, ab, kt, i, h]
                w1big = wpool.tile(
                    [128, 2, KT8, 2, H], F8, tag="w1big", name="w1big"
                )
                w2t = wpool.tile([128, 2, H], F16, tag="w2t", name="w2t")
                w3t = wpool.tile([128, 2, H], F32, tag="w3t", name="w3t")
                bp = wpool.tile([128, 6], F32, tag="bp", name="bp")
                idt = wpool.tile([128, 2, 128], F8, tag="idt", name="idt")
                # y per half: [p, m(4)=(ya0,ya1,yb0,yb1), c, b]
                ysb = [
                    ypool.tile([128, 4, C, BH], F16, tag=f"y{h}", name=f"y{h}")
                    for h in range(NH)
                ]
                # h1 per half: [p, t(2), pair, b]
                h1sb = [
                    hpool.tile([128, 2, P, BH], F16, tag=f"h1_{h}", name=f"h1_{h}")
                    for h in range(NH)
                ]
                # h2 per (m): [p, half, group, col], fp8 so the pair-sum can
                # run as DoubleRow identity matmuls (precision cost ~3e-4:
                # fp8 noise averages out over the 120-pair sum)
                h2sb = [
                    spool.tile(
                        [128, NH, NG, GW], F8, tag=f"h2_{m}", name=f"h2_{m}"
                    )
                    for m in range(2)
                ]
                m2 = [
                    spool.tile([128, BL], F32, tag=f"m2_{m}", name=f"m2_{m}")
                    for m in range(2)
                ]
                osb = spool.tile([128, 2, BL], F32, tag="osb", name="osb")

                def bias(nm, t):
                    idx = {"b1": 0, "b2": 2, "b3": 4}[nm] + t
                    return bp[:, idx : idx + 1]

                # flat views for DMA (<=3 AP dims per transfer)
                w1flat = w1big.rearrange("q ab kt i h -> q (ab kt i h)")
                xflat = xts.rearrange("q kt i t -> q (kt i) t")
                xdv = xt_d.rearrange("q (ki t) -> q ki t", t=TOK)

                def hs(half):
                    return slice(half * HTOK, (half + 1) * HTOK)

                # PE warm-up on a memset scratch tile: burns the p-state ramp
                # from t~0 without waiting for any DMA
                scr = wpool.tile([128, 4], F8, tag="scr", name="scr")
                nc.vector.memset(scr[:], 0.0)
                warm = psC_pool.tile([128, 1024], F32, tag="psC", name="warm")
                for _ in range(N_WARM):
                    nc.tensor.matmul(
                        warm[:1, :256],
                        scr[:, 0:1],
                        scr[:, 0:1].broadcast_to([128, 256]),
                        start=True,
                        stop=True,
                    )

                # ---- DMA order. Stage-A's critical path (W1a + first half of
                # x h0) goes on the sync/HWDGE queue for fast start; the rest
                # streams via SWDGE. ----
                # All loads go on the sync/HWDGE queue: the shared DMA engine
                # grants transfers in issue-ready order, and a single queue
                # is the only way to pin the wire order to stage-A's
                # consumption order.
                nc.sync.dma_start(
                    out=w1flat[:, : KT8 * 2 * H], in_=w1_d[:, : KT8 * 2 * H]
                )
                nc.sync.dma_start(
                    out=xflat[:, 0:4, hs(0)], in_=xdv[:, 0:4, hs(0)]
                )
                nc.sync.dma_start(
                    out=xflat[:, 4:8, hs(0)], in_=xdv[:, 4:8, hs(0)]
                )
                nc.sync.dma_start(
                    out=w1flat[:, KT8 * 2 * H :], in_=w1_d[:, KT8 * 2 * H :]
                )
                nc.sync.dma_start(
                    out=xflat[:, 0:4, hs(1)], in_=xdv[:, 0:4, hs(1)]
                )
                nc.sync.dma_start(
                    out=xflat[:, 4:8, hs(1)], in_=xdv[:, 4:8, hs(1)]
                )
                nc.sync.dma_start(out=bp[:], in_=bp_d[:])
                nc.sync.dma_start(
                    out=idt.rearrange("q i m -> q (i m)"), in_=id_d[:]
                )
                nc.sync.dma_start(
                    out=w2t[:], in_=w2_d.rearrange("(k p) h -> p k h", p=128)
                )
                nc.sync.dma_start(
                    out=w3t[:], in_=w3_d.rearrange("(k p) h -> p k h", p=128)
                )

                # pair-add instruction i covers pairs PAIR_BASE[i]..+(C-1-i);
                # stage-C group g needs all i-instrs whose range starts
                # before pair 30*(g+1)
                i_done = []
                acc = 0
                for g in range(NG):
                    need = PPG * (g + 1)
                    i_hi = acc
                    while i_hi < C - 1 and PAIR_BASE[i_hi] < need:
                        i_hi += 1
                    i_done.append(i_hi)
                    acc = i_hi

                # ---- stage A, both halves back-to-back on PE (A(h1) fills
                # the pair-add/relu latency of half 0). Half-0 evictions run
                # immediately (ya on ACT, yb on DVE); half-1 evictions are
                # DEFERRED and emitted on DVE at the start of half 1's
                # pair-add section - both engines execute in order, so an
                # early-emitted h1 evict would head-of-line block the h0
                # pair-add chain. ----
                deferred_evicts = []
                for half in range(NH):
                    yv = ysb[half].rearrange("q m c b -> q m (c b)")
                    for ab in range(2):
                        for ht in range(2):
                            m = ab * 2 + ht
                            psa = psA_pool.tile(
                                [128, HTOK], F32, tag="psA",
                                name=f"psa_{half}_{m}",
                            )
                            for kt in range(KT8):
                                nc.tensor.matmul(
                                    psa[:],
                                    w1big[:, ab, kt, :, ht * 128 : (ht + 1) * 128],
                                    xts[:, kt, :, hs(half)],
                                    start=(kt == 0),
                                    stop=(kt == KT8 - 1),
                                    perf_mode=DR,
                                )
                            # A-evict: fp32->fp16 copy with 1/W1SCALE descale
                            if half == 0:
                                if ab == 0:
                                    nc.scalar.activation(
                                        yv[:, m, :], psa[:], AF.Copy,
                                        scale=1.0 / W1SCALE,
                                    )
                                else:
                                    nc.vector.tensor_scalar_mul(
                                        yv[:, m, :], psa[:], 1.0 / W1SCALE
                                    )
                            else:
                                deferred_evicts.append((yv[:, m, :], psa))

                # preload the Relu activation table (emitted after the h0
                # A-evicts so it doesn't head-of-line block them on ACT)
                nc.scalar.activation(osb[:1, 0, 0:1], bp[:1, 0:1], AF.Relu)
                # half-1 A-evicts run in ACT's idle window here - putting
                # them on DVE would lengthen the serial DVE chain
                # (pair-adds/relus) that paces the kernel middle
                for out_ap, psa in deferred_evicts:
                    nc.scalar.activation(
                        out_ap, psa[:], AF.Copy, scale=1.0 / W1SCALE
                    )
                deferred_evicts = []

                # ---- stages B+C, pipelined at 30-pair groups; the pair-sum
                # flush matmuls are emitted lagging 2 (g, m)-steps behind so
                # the PE never waits on an ACT eviction ----
                psr = {}
                flush_q = []
                reduces_done = {0: 0, 1: 0}

                def emit_flush():
                    fhalf, fg, fm = flush_q.pop(0)
                    nc.tensor.matmul(
                        psr[(fhalf, fm)][:, :SUBW],
                        idt[:],
                        h2sb[fm][:, fhalf, fg, :].rearrange(
                            "q (i w) -> q i w", i=2
                        ),
                        start=(fg == 0),
                        stop=(fg == NG - 1),
                        perf_mode=DR,
                    )
                    if fg == NG - 1:
                        # pair-sum reduce: psr [q, (15 p, b)] -> m2 col block
                        v = psr[(fhalf, fm)].rearrange(
                            "q (pp b) -> q pp b", b=BH
                        ).transpose([0, 2, 1])
                        nc.vector.tensor_reduce(
                            m2[fm][:, fhalf * BH : (fhalf + 1) * BH],
                            v[:, :, : PPG // 2],
                            AX.X,
                            ALU.add,
                        )
                        reduces_done[fhalf] += 1
                        if reduces_done[fhalf] == 2:
                            emit_E(fhalf)

                def emit_E(half):
                    # stage E for this half's batch columns:
                    # outT[:, half] = m2[:, half] @ (W3/P) + b3
                    bsl = slice(half * BH, (half + 1) * BH)
                    for mo in range(2):
                        psE = psA_pool.tile(
                            [128, HTOK], F32, tag="psA", name=f"psE_{half}_{mo}"
                        )
                        po = psE[:, :BH]
                        for k in range(2):
                            nc.tensor.matmul(
                                po,
                                w3t[:, k, mo * 128 : (mo + 1) * 128],
                                m2[k][:, bsl],
                                start=(k == 0),
                                stop=(k == 1),
                            )
                        nc.vector.tensor_scalar_add(
                            osb[:, mo, bsl], po, bias("b3", mo)
                        )
                    nc.sync.dma_start(
                        out=out_d.rearrange("(m p) b -> p m b", p=128)[
                            :, :, bsl
                        ],
                        in_=osb[:, :, bsl],
                    )

                for half in range(NH):
                    i_issued = 0
                    h1v = h1sb[half]
                    for g in range(NG):
                        # pair-adds needed for this group
                        for i in range(i_issued, i_done[g]):
                            nj = C - 1 - i
                            p0 = PAIR_BASE[i]
                            nc.vector.tensor_add(
                                h1v[:, :, p0 : p0 + nj, :],
                                ysb[half][:, 0:2, i : i + 1, :].broadcast_to(
                                    [128, 2, nj, BH]
                                ),
                                ysb[half][:, 2:4, i + 1 :, :],
                            )
                        i_issued = i_done[g]
                        # relu + b1 in place (per t)
                        for t in range(2):
                            sl = h1v[:, t, g * PPG : (g + 1) * PPG, :]
                            nc.vector.tensor_scalar(
                                sl, sl, bias("b1", t), 0.0, ALU.add, ALU.max
                            )
                        # layer-2 matmuls + evict; flush lags 2 steps
                        for m in range(2):
                            if (half, m) not in psr:
                                psr[(half, m)] = psR_pool.tile(
                                    [128, HTOK], F32, tag="psR",
                                    name=f"psr_{half}_{m}",
                                )
                            ps = psC_pool.tile(
                                [128, 1024], F32, tag="psC",
                                name=f"psC_{half}_{g}_{m}",
                            )
                            for sub in range(2):
                                for k in range(2):
                                    nc.tensor.matmul(
                                        ps[:, sub * 512 : sub * 512 + SUBW],
                                        w2t[:, k, m * 128 : (m + 1) * 128],
                                        h1v[
                                            :,
                                            k,
                                            g * PPG
                                            + sub * (PPG // 2) : g * PPG
                                            + (sub + 1) * (PPG // 2),
                                            :,
                                        ],
                                        start=(k == 0),
                                        stop=(k == 1),
                                    )
                            h2t = h2sb[m][:, half, g, :].rearrange(
                                "q (u w) -> q u w", w=SUBW
                            )
                            psv = ps.rearrange("q (u w) -> q u w", w=512)[
                                :, :, :SUBW
                            ]
                            nc.scalar.activation(
                                h2t, psv, AF.Relu, bias=bias("b2", m)
                            )
                            flush_q.append((half, g, m))
                            if len(flush_q) > 2:
                                emit_flush()
                while flush_q:
                    emit_flush()

    nc.compile()
    return nc


_NC_CACHE = None


def _get_module():
    global _NC_CACHE
    if _NC_CACHE is None:
        _NC_CACHE = build_module()
    return _NC_CACHE


def make_in_maps(x, W1, b1, W2, b2, W3, b3):
    e4 = ml_dtypes.float8_e4m3
    # W1 [2F, H] -> [p, ab, kt, i, h] flat, prescaled for fp8 mantissa range
    w1_8 = (np.asarray(W1, dtype=np.float32) * W1SCALE).astype(e4)
    w1_8 = np.ascontiguousarray(
        w1_8.reshape(2, KT8, 2, 128, H).transpose(3, 0, 1, 2, 4).reshape(128, -1)
    )
    w2_h = np.ascontiguousarray(np.asarray(W2, dtype=np.float32).astype(np.float16))
    w3p = np.ascontiguousarray(np.asarray(W3, dtype=np.float32) / np.float32(P))
    b1 = np.asarray(b1, dtype=np.float32)
    b2 = np.asarray(b2, dtype=np.float32)
    b3 = np.asarray(b3, dtype=np.float32)
    bias_pack = np.stack(
        [b1[:128], b1[128:], b2[:128], b2[128:], b3[:128], b3[128:]], axis=1
    )
    bias_pack = np.ascontiguousarray(bias_pack, dtype=np.float32)
    ident2 = np.ascontiguousarray(
        np.stack([np.eye(128), np.eye(128)], axis=1).reshape(128, 256)
    ).astype(e4)
    x = np.asarray(x, dtype=np.float32)
    in_maps = []
    for ci in range(N_CORES):
        xs = x[ci * BL : (ci + 1) * BL]  # [BL, C, F]
        halves = [
            xs[h * BH : (h + 1) * BH].transpose(1, 0, 2).reshape(HTOK, F)
            for h in range(NH)
        ]
        xT = np.concatenate(halves, axis=0).T  # [F, TOK]
        # [F, TOK] -> [p, kt, i, t] flat with f = kt*256 + i*128 + p
        xT8 = (
            xT.astype(e4)
            .reshape(KT8, 2, 128, TOK)
            .transpose(2, 0, 1, 3)
            .reshape(128, -1)
        )
        in_maps.append(
            {
                "xt": np.ascontiguousarray(xT8),
                "w1": w1_8,
                "w2": w2_h,
                "w3": w3p,
                "bias_pack": bias_pack,
                "ident2": ident2,
            }
        )
    return in_maps


def kernel(x, W1, b1, W2, b2, W3, b3):
    nc = _get_module()
    in_maps = make_in_maps(x, W1, b1, W2, b2, W3, b3)
    res = run_bass_kernel_spmd(nc, in_maps, list(range(N_CORES)))
    out = np.empty((B, H), dtype=np.float32)
    for i in range(N_CORES):
        out[i * BL : (i + 1) * BL] = res.results[i]["outT"].T
    return out
